# revision 17
# baseline (speedup 1.0000x reference)
"""Trainium2 Bass kernel for NodeAttention-style pooling.

Math (the reference's two linear layers have no nonlinearity between them,
so they collapse; the bias terms are constant over the softmax axis and
cancel in U/Z):
    score[b,s,v] = x[b,s,v,:] . weff          with weff = (W2 @ W1)[0]
    e = exp(score)                             (fp16 on device)
    U[b,v,:] = sum_s e[b,s,v] * x[b,s,v,:]    (unnormalized, device)
    Z[b,v]   = sum_s exp(score[b,s,v])        (host, from score shipped out)
    out = U / Z                                (host divide)

Sharding: vocab axis V=1024 split 128-per-core across 8 cores (softmax and
pooling are independent per (b, v) — no communication).

Per-core design (x shard = 64 MiB f32, HBM roofline ~188 us; every engine
must fit under that):
  - scores run as ONE custom-DVE instruction per half-chunk (MUL_SCAN:
    out = running prefix sum of x*weff along the free dim). Per-vocab dot
    products are differences of prefix samples at 512-element boundaries,
    extracted with one strided tensor_sub. DVE: ~8.8 us/chunk.
  - f32->fp16 conversion of x (needed for the PE weighted sum; fp32 matmul
    is 4 cyc/row) runs on ACT, which otherwise only does the small exp and
    the PSUM->SBUF staging copy: ~9.6 us/chunk.
  - the weighted sum stays on the PE: M=1 matmuls with exp-weights as the
    1-column stationary, tile_position col-groups packing 4 outputs per
    PSUM bank (partitions 0/32/64/96). No normalization on device, so no
    transposes, no reciprocal, no ACT accumulator reads.
  - ACT's in-order queue is software-pipelined: chunk i's staging copy is
    emitted between chunk i+1's conversions so the long per-chunk
    dependency chain never serializes the engine.
  - the globally-last chunk runs at quarter granularity (4 vocab rows =
    one PSUM col-group per quarter) to shrink the post-DMA tail chain.
"""

import numpy as np

B, S, V, D = 2, 128, 1024, 512
NCORES = 8
VS = V // NCORES  # 128 vocab entries per core
VC = 16           # vocab entries per chunk
NCHUNK = VS // VC
NGRP = VC // 4    # psum col-group packs per chunk
P = 128
HALF = VC // 2    # vocab rows per half-chunk
QUAR = 4          # vocab rows per quarter (tail chunk only)

_NC_CACHE = {}


def _make_mul_scan():
    """Register the MUL_SCAN custom DVE op (prefix sum of Src0*Src1)."""
    import concourse.dve_ops as dve_ops
    from concourse.dve_spec import Spec, Src0, Src1, AluOp, scan, lower
    from concourse.dve_uop import DveOpSpec

    for op in dve_ops.OPS:
        if op.name == "MUL_SCAN":
            return op

    def ref(in0, in1, s0, s1, imm2):
        p = in0.shape[0]
        prod = (np.asarray(in0, np.float32) * np.asarray(in1, np.float32)).reshape(
            p, -1
        )
        return np.cumsum(prod, axis=1, dtype=np.float32).reshape(in0.shape)

    spec = Spec(body=scan(AluOp.ADD, Src0 * Src1), reference=ref)
    row = dve_ops._CUSTOM_DVE_ROW_BASE + len(dve_ops.OPS)
    assert row < 0x20
    shas = {}
    for ver in ("v3", "v4"):
        tmp = DveOpSpec(name="MUL_SCAN", opcode=row, uops=lower(spec, ver=ver),
                        rd1_en=True)
        shas[ver] = tmp.sha(ver)
    op = dve_ops.DveOp("MUL_SCAN", spec, subdim=False, uops_sha=shas)
    dve_ops.OPS.append(op)
    dve_ops.CUSTOM_DVE_SPECS[op.name] = op.spec
    dve_ops._SUB_OPCODE_FOR_NAME[op.name] = row
    return op


def build_nc():
    import concourse.bacc as bacc
    import concourse.tile as tile
    from concourse import mybir

    MUL_SCAN = _make_mul_scan()

    f32 = mybir.dt.float32
    f16 = mybir.dt.float16
    nc = bacc.Bacc(
        "TRN2",
        target_bir_lowering=False,
        debug=False,
        enable_asserts=False,
        num_devices=NCORES,
    )

    x_h = nc.dram_tensor("x", [B, S, VS, D], f32, kind="ExternalInput")
    wb_h = nc.dram_tensor("weffb", [P, D], f32, kind="ExternalInput")
    out_h = nc.dram_tensor("out", [B, 1, VS * D], f32, kind="ExternalOutput")
    sc_h = nc.dram_tensor("sc", [B, NCHUNK, S, VC], f32, kind="ExternalOutput")
    x = x_h.ap()
    wb = wb_h.ap()
    out = out_h.ap()
    scout = sc_h.ap()

    with tile.TileContext(nc) as tc:
        with (
            tc.tile_pool(name="singles", bufs=1) as singles,
            tc.tile_pool(name="chunks", bufs=4) as chunks,
            tc.tile_pool(name="chunk16p", bufs=2) as chunk16p,
            tc.tile_pool(name="prefp", bufs=1) as prefp,
            tc.tile_pool(name="scp", bufs=2) as scp,
            tc.tile_pool(name="e16p", bufs=2) as e16p,
            tc.tile_pool(name="stagep", bufs=2) as stagep,
            tc.tile_pool(name="bankp", bufs=1, space="PSUM") as bankp,
        ):
            wb_t = singles.tile([P, D], f32, name="wb_t")

            # Two alternating 4-bank PSUM tiles for the weighted-sum outputs
            # (so chunk i+1's matmuls never wait on chunk i's staging copy);
            # zeroed once so the junk-row ACT copies never see non-float
            # bit patterns.
            bigbanks = []
            for k in range(2):
                bb = bankp.tile([P, NGRP, D], f32, name=f"bigbank{k}")
                nc.vector.memset(bb, 0.0)
                bigbanks.append(bb)

            pending = [None]

            def flush_pending():
                if pending[0] is not None:
                    pending[0]()
                    pending[0] = None

            def scan_rows(src, n_rows, sct_slice):
                """Scores for `n_rows` vocab rows: fused mul+prefix-scan,
                then one strided diff. Returns nothing; writes sct_slice."""
                pp = prefp.tile([P, n_rows * D + 1], f32, name="pp",
                                tag="pp")
                nc.vector.memset(pp[:, 0:1], 0.0)
                nc.vector._custom_dve(
                    MUL_SCAN,
                    out=pp[:, 1 : n_rows * D + 1].rearrange(
                        "p (r d) -> p r d", d=D
                    ),
                    in0=src,
                    in1=wb_t.unsqueeze(1).broadcast_to((P, n_rows, D)),
                )
                nc.vector.tensor_sub(
                    sct_slice,
                    pp[:, D :: D],
                    pp[:, 0 :: D][:, :n_rows],
                )

            first = True
            for b in range(B):
                for ci in range(NCHUNK):
                    v0 = ci * VC
                    gi = b * NCHUNK + ci
                    last_chunk = gi == B * NCHUNK - 1
                    penult = gi == B * NCHUNK - 2
                    # last chunk: quarters shorten the post-DMA tail
                    quartered = last_chunk
                    nparts = 4 if quartered else 2
                    rows = VC // nparts

                    parts = []
                    for h in range(nparts):
                        ch = chunks.tile([P, rows, D], f32, name=f"chunk{h}",
                                         tag=f"chunk{h % 2}")
                        nc.sync.dma_start(
                            out=ch,
                            in_=x[b, :, v0 + h * rows : v0 + (h + 1) * rows, :],
                        )
                        parts.append(ch)
                    if first:
                        # weights load ordered after the first x triggers so
                        # it never delays the long pole
                        nc.sync.dma_start(out=wb_t, in_=wb)
                        first = False

                    chunk16 = chunk16p.tile([P, VC, D], f16, name="chunk16")
                    sct = scp.tile([P, VC], f32, name="sct")
                    e16 = e16p.tile([P, VC], f16, name="e16")

                    def do_part(h, conv_on_dve=False):
                        # f32 -> fp16 for the PE; normally on ACT, but the
                        # tail chunk keeps ACT free for exp/staging
                        dst = chunk16[:, h * rows : (h + 1) * rows, :]
                        if conv_on_dve:
                            nc.vector.tensor_copy(dst, parts[h])
                        else:
                            nc.scalar.copy(dst, parts[h])
                        scan_rows(parts[h], rows,
                                  sct[:, h * rows : (h + 1) * rows])

                    def do_exp(h):
                        nc.scalar.activation(
                            out=e16[:, h * rows : (h + 1) * rows],
                            in_=sct[:, h * rows : (h + 1) * rows],
                            func=mybir.ActivationFunctionType.Exp,
                        )

                    def do_mms(h, bigbank):
                        for g in range(h * rows // 4, (h + 1) * rows // 4):
                            for j in range(4):
                                vl = g * 4 + j
                                nc.tensor.matmul(
                                    bigbank[32 * j : 32 * j + 1, g, :],
                                    lhsT=e16[:, vl : vl + 1],
                                    rhs=chunk16[:, vl, :],
                                    tile_position=(0, 32 * j),
                                )

                    if not quartered:
                        bigbank = bigbanks[gi % 2]
                        do_part(0)
                        do_exp(0)
                        # chunk i-1's staging copy + output DMAs slot in
                        # here so ACT's in-order queue stays pipelined
                        flush_pending()
                        do_part(1)
                        do_mms(0, bigbank)
                        do_exp(1)
                        do_mms(1, bigbank)

                        def emit_stag(b=b, ci=ci, v0=v0, sct=sct,
                                      bigbank=bigbank, split=penult):
                            # the chunk before the tail splits its staging
                            # copy so it never blocks the tail's ACT work
                            nsp = 2 if split else 1
                            gg = NGRP // nsp
                            for sp in range(nsp):
                                stag = stagep.tile([P, gg * D], f32,
                                                   name="stag", tag="stag")
                                nc.scalar.copy(
                                    stag[0:97, :],
                                    bigbank[
                                        0:97, sp * gg : (sp + 1) * gg, :
                                    ].rearrange("p g d -> p (g d)"),
                                )
                                src = stag.rearrange(
                                    "(g r) n -> g r n", r=32
                                )[:, 0, :].rearrange(
                                    "j (k d) -> j k d", d=D
                                )
                                lo = (v0 + sp * gg * 4) * D
                                hi = (v0 + (sp + 1) * gg * 4) * D
                                dst = out[b, :, lo:hi].rearrange(
                                    "o (k j d) -> o j k d", j=4, d=D
                                )[0]
                                nc.sync.dma_start(out=dst, in_=src)
                            nc.sync.dma_start(out=scout[b, ci], in_=sct)

                        pending[0] = emit_stag
                    else:
                        # quartered chunk: each quarter is one PSUM group,
                        # alternating PSUM tiles so matmuls never wait on
                        # the previous quarter's staging copy
                        do_part(0, conv_on_dve=last_chunk)
                        do_exp(0)
                        flush_pending()
                        for h in range(nparts):
                            if h > 0:
                                do_part(h, conv_on_dve=last_chunk)
                                do_exp(h)
                            bigbank = bigbanks[h % 2]
                            do_mms(h, bigbank)
                            g = h  # quarter h == psum group h
                            stag = stagep.tile([P, D], f32, name="stagq",
                                               tag="stag")
                            nc.scalar.copy(stag[0:97, :], bigbank[0:97, g, :])
                            src = stag.rearrange("(g r) n -> g r n", r=32)[
                                :, 0, :
                            ]
                            dst = out[
                                b, :,
                                (v0 + g * 4) * D : (v0 + (g + 1) * 4) * D,
                            ].rearrange("o (j d) -> o j d", d=D)[0]
                            nc.sync.dma_start(out=dst, in_=src)
                        nc.sync.dma_start(out=scout[b, ci], in_=sct)
            flush_pending()

    nc.compile()
    return nc


def _get_nc():
    if "nc" not in _NC_CACHE:
        _NC_CACHE["nc"] = build_nc()
    return _NC_CACHE["nc"]


def _host_prep(x, W1, b1, W2, b2):
    x = np.ascontiguousarray(np.asarray(x, dtype=np.float32))
    W1 = np.asarray(W1, dtype=np.float64)
    W2 = np.asarray(W2, dtype=np.float64)
    weff = (W2 @ W1)[0].astype(np.float32)  # [D]
    weffb = np.ascontiguousarray(np.broadcast_to(weff, (P, D)))
    in_maps = []
    for c in range(NCORES):
        shard = np.ascontiguousarray(x[:, :, c * VS : (c + 1) * VS, :])
        in_maps.append({"x": shard, "weffb": weffb})
    return in_maps


def _host_post(results):
    """Divide the unnormalized pooled sums by Z computed from the scores."""
    outs = []
    for r in results:
        U = r["out"].reshape(B, VS, D).astype(np.float64)
        sc = r["sc"].astype(np.float64)  # [B, NCHUNK, S, VC]
        Z = np.exp(sc).sum(axis=2).reshape(B, VS)  # [B, VS]
        outs.append((U / Z[..., None]).astype(np.float32))
    return np.concatenate(outs, axis=1)


def kernel(x, W1, b1, W2, b2):
    from concourse.bass_utils import run_bass_kernel_spmd

    in_maps = _host_prep(x, W1, b1, W2, b2)
    nc = _get_nc()
    res = run_bass_kernel_spmd(nc, in_maps, core_ids=list(range(NCORES)))
    return _host_post(res.results)


# revision 18
# speedup vs baseline: 1.2181x; 1.2181x over previous
"""Trainium2 Bass kernel for NodeAttention-style pooling.

Math (the reference's two linear layers have no nonlinearity between them,
so they collapse; the bias terms are constant over the softmax axis and
cancel in U/Z):
    score[b,s,v] = x[b,s,v,:] . weff          with weff = (W2 @ W1)[0]
    e = exp(score)                             (fp16 on device)
    U[b,v,:] = sum_s e[b,s,v] * x[b,s,v,:]    (unnormalized, device)
    Z[b,v]   = sum_s exp(score[b,s,v])        (host, from score shipped out)
    out = U / Z                                (host divide)

Sharding: vocab axis V=1024 split 128-per-core across 8 cores (softmax and
pooling are independent per (b, v) — no communication).

Per-core design (x shard = 64 MiB f32, HBM roofline ~188 us; every engine
must fit under that):
  - scores run as ONE custom-DVE instruction per half-chunk (MUL_SCAN:
    out = running prefix sum of x*weff along the free dim). Per-vocab dot
    products are differences of prefix samples at 512-element boundaries,
    extracted with one strided tensor_sub. DVE: ~8.8 us/chunk.
  - f32->fp16 conversion of x (needed for the PE weighted sum; fp32 matmul
    is 4 cyc/row) runs on ACT, which otherwise only does the small exp and
    the PSUM->SBUF staging copy: ~9.6 us/chunk.
  - the weighted sum stays on the PE: M=1 matmuls with exp-weights as the
    1-column stationary, tile_position col-groups packing 4 outputs per
    PSUM bank (partitions 0/32/64/96). No normalization on device, so no
    transposes, no reciprocal, no ACT accumulator reads.
  - ACT's in-order queue is software-pipelined: chunk i's staging copy is
    emitted between chunk i+1's conversions so the long per-chunk
    dependency chain never serializes the engine.
  - the globally-last chunk runs at quarter granularity (4 vocab rows =
    one PSUM col-group per quarter) to shrink the post-DMA tail chain.
"""

import numpy as np

B, S, V, D = 2, 128, 1024, 512
NCORES = 8
VS = V // NCORES  # 128 vocab entries per core
VC = 16           # vocab entries per chunk
NCHUNK = VS // VC
NGRP = VC // 4    # psum col-group packs per chunk
P = 128
HALF = VC // 2    # vocab rows per half-chunk
QUAR = 4          # vocab rows per quarter (tail chunk only)

_NC_CACHE = {}


def _make_mul_scan():
    """Register the MUL_SCAN custom DVE op (prefix sum of Src0*Src1)."""
    import concourse.dve_ops as dve_ops
    from concourse.dve_spec import Spec, Src0, Src1, AluOp, scan, lower
    from concourse.dve_uop import DveOpSpec

    for op in dve_ops.OPS:
        if op.name == "MUL_SCAN":
            return op

    def ref(in0, in1, s0, s1, imm2):
        p = in0.shape[0]
        prod = (np.asarray(in0, np.float32) * np.asarray(in1, np.float32)).reshape(
            p, -1
        )
        return np.cumsum(prod, axis=1, dtype=np.float32).reshape(in0.shape)

    spec = Spec(body=scan(AluOp.ADD, Src0 * Src1), reference=ref)
    row = dve_ops._CUSTOM_DVE_ROW_BASE + len(dve_ops.OPS)
    assert row < 0x20
    shas = {}
    for ver in ("v3", "v4"):
        tmp = DveOpSpec(name="MUL_SCAN", opcode=row, uops=lower(spec, ver=ver),
                        rd1_en=True)
        shas[ver] = tmp.sha(ver)
    op = dve_ops.DveOp("MUL_SCAN", spec, subdim=False, uops_sha=shas)
    dve_ops.OPS.append(op)
    dve_ops.CUSTOM_DVE_SPECS[op.name] = op.spec
    dve_ops._SUB_OPCODE_FOR_NAME[op.name] = row
    return op


def build_nc():
    import concourse.bacc as bacc
    import concourse.tile as tile
    from concourse import mybir

    MUL_SCAN = _make_mul_scan()

    f32 = mybir.dt.float32
    f16 = mybir.dt.float16
    nc = bacc.Bacc(
        "TRN2",
        target_bir_lowering=False,
        debug=False,
        enable_asserts=False,
        num_devices=NCORES,
    )

    x_h = nc.dram_tensor("x", [B, S, VS, D], f32, kind="ExternalInput")
    wb_h = nc.dram_tensor("weffb", [P, D], f32, kind="ExternalInput")
    out_h = nc.dram_tensor("out", [B, 1, VS * D], f32, kind="ExternalOutput")
    sc_h = nc.dram_tensor("sc", [B, NCHUNK, S, VC], f32, kind="ExternalOutput")
    x = x_h.ap()
    wb = wb_h.ap()
    out = out_h.ap()
    scout = sc_h.ap()

    with tile.TileContext(nc) as tc:
        with (
            tc.tile_pool(name="singles", bufs=1) as singles,
            tc.tile_pool(name="chunks", bufs=4) as chunks,
            tc.tile_pool(name="chunk16p", bufs=2) as chunk16p,
            tc.tile_pool(name="prefp", bufs=1) as prefp,
            tc.tile_pool(name="scp", bufs=2) as scp,
            tc.tile_pool(name="e16p", bufs=2) as e16p,
            tc.tile_pool(name="stagep", bufs=2) as stagep,
            tc.tile_pool(name="bankp", bufs=1, space="PSUM") as bankp,
        ):
            wb_t = singles.tile([P, D], f32, name="wb_t")

            # Two alternating 4-bank PSUM tiles for the weighted-sum outputs
            # (so chunk i+1's matmuls never wait on chunk i's staging copy);
            # zeroed once so the junk-row ACT copies never see non-float
            # bit patterns.
            bigbanks = []
            for k in range(2):
                bb = bankp.tile([P, NGRP, D], f32, name=f"bigbank{k}")
                nc.vector.memset(bb, 0.0)
                bigbanks.append(bb)

            pending = [None]

            def flush_pending():
                if pending[0] is not None:
                    pending[0]()
                    pending[0] = None

            def scan_rows(src, n_rows, sct_slice):
                """Scores for `n_rows` vocab rows: fused mul+prefix-scan,
                then one strided diff. Returns nothing; writes sct_slice."""
                pp = prefp.tile([P, n_rows * D + 1], f32, name="pp",
                                tag="pp")
                nc.vector.memset(pp[:, 0:1], 0.0)
                nc.vector._custom_dve(
                    MUL_SCAN,
                    out=pp[:, 1 : n_rows * D + 1].rearrange(
                        "p (r d) -> p r d", d=D
                    ),
                    in0=src,
                    in1=wb_t.unsqueeze(1).broadcast_to((P, n_rows, D)),
                )
                nc.vector.tensor_sub(
                    sct_slice,
                    pp[:, D :: D],
                    pp[:, 0 :: D][:, :n_rows],
                )

            first = True
            for b in range(B):
                for ci in range(NCHUNK):
                    v0 = ci * VC
                    gi = b * NCHUNK + ci
                    last_chunk = gi == B * NCHUNK - 1
                    penult = gi == B * NCHUNK - 2
                    # last chunk: quarters shorten the post-DMA tail
                    quartered = last_chunk
                    nparts = 4 if quartered else 2
                    rows = VC // nparts

                    parts = []
                    for h in range(nparts):
                        ch = chunks.tile([P, rows, D], f32, name=f"chunk{h}",
                                         tag=f"chunk{h % 2}")
                        nc.sync.dma_start(
                            out=ch,
                            in_=x[b, :, v0 + h * rows : v0 + (h + 1) * rows, :],
                        )
                        parts.append(ch)
                    if first:
                        # weights load ordered after the first x triggers so
                        # it never delays the long pole
                        nc.sync.dma_start(out=wb_t, in_=wb)
                        first = False

                    chunk16 = chunk16p.tile([P, VC, D], f16, name="chunk16")
                    sct = scp.tile([P, VC], f32, name="sct")
                    e16 = e16p.tile([P, VC], f16, name="e16")

                    def do_part(h, conv_on_dve=False):
                        # f32 -> fp16 for the PE; normally on ACT, but the
                        # tail chunk keeps ACT free for exp/staging
                        dst = chunk16[:, h * rows : (h + 1) * rows, :]
                        if conv_on_dve:
                            nc.vector.tensor_copy(dst, parts[h])
                        else:
                            nc.scalar.copy(dst, parts[h])
                        scan_rows(parts[h], rows,
                                  sct[:, h * rows : (h + 1) * rows])

                    def do_exp(h):
                        nc.scalar.activation(
                            out=e16[:, h * rows : (h + 1) * rows],
                            in_=sct[:, h * rows : (h + 1) * rows],
                            func=mybir.ActivationFunctionType.Exp,
                        )

                    def do_mms(h, bigbank):
                        for g in range(h * rows // 4, (h + 1) * rows // 4):
                            for j in range(4):
                                vl = g * 4 + j
                                nc.tensor.matmul(
                                    bigbank[32 * j : 32 * j + 1, g, :],
                                    lhsT=e16[:, vl : vl + 1],
                                    rhs=chunk16[:, vl, :],
                                    tile_position=(0, 32 * j),
                                )

                    if not quartered:
                        bigbank = bigbanks[gi % 2]
                        do_part(0)
                        do_exp(0)
                        # chunk i-1's staging copy + output DMAs slot in
                        # here so ACT's in-order queue stays pipelined
                        flush_pending()
                        do_part(1)
                        do_mms(0, bigbank)
                        do_exp(1)
                        do_mms(1, bigbank)

                        def emit_stag(b=b, ci=ci, v0=v0, sct=sct,
                                      bigbank=bigbank, split=penult):
                            # the chunk before the tail splits its staging
                            # copy so it never blocks the tail's ACT work
                            nsp = 2 if split else 1
                            gg = NGRP // nsp
                            for sp in range(nsp):
                                stag = stagep.tile([P, gg * D], f32,
                                                   name="stag", tag="stag")
                                nc.scalar.copy(
                                    stag[0:97, :],
                                    bigbank[
                                        0:97, sp * gg : (sp + 1) * gg, :
                                    ].rearrange("p g d -> p (g d)"),
                                )
                                src = stag.rearrange(
                                    "(g r) n -> g r n", r=32
                                )[:, 0, :].rearrange(
                                    "j (k d) -> j k d", d=D
                                )
                                lo = (v0 + sp * gg * 4) * D
                                hi = (v0 + (sp + 1) * gg * 4) * D
                                dst = out[b, :, lo:hi].rearrange(
                                    "o (k j d) -> o j k d", j=4, d=D
                                )[0]
                                nc.gpsimd.dma_start(out=dst, in_=src)
                            nc.gpsimd.dma_start(out=scout[b, ci], in_=sct)

                        pending[0] = emit_stag
                    else:
                        # quartered chunk: each quarter is one PSUM group,
                        # alternating PSUM tiles so matmuls never wait on
                        # the previous quarter's staging copy
                        do_part(0, conv_on_dve=last_chunk)
                        do_exp(0)
                        flush_pending()
                        for h in range(nparts):
                            if h > 0:
                                do_part(h, conv_on_dve=last_chunk)
                                do_exp(h)
                            bigbank = bigbanks[h % 2]
                            do_mms(h, bigbank)
                            g = h  # quarter h == psum group h
                            stag = stagep.tile([P, D], f32, name="stagq",
                                               tag="stag")
                            nc.scalar.copy(stag[0:97, :], bigbank[0:97, g, :])
                            src = stag.rearrange("(g r) n -> g r n", r=32)[
                                :, 0, :
                            ]
                            dst = out[
                                b, :,
                                (v0 + g * 4) * D : (v0 + (g + 1) * 4) * D,
                            ].rearrange("o (j d) -> o j d", d=D)[0]
                            nc.gpsimd.dma_start(out=dst, in_=src)
                        nc.gpsimd.dma_start(out=scout[b, ci], in_=sct)
            flush_pending()

    nc.compile()
    return nc


def _get_nc():
    if "nc" not in _NC_CACHE:
        _NC_CACHE["nc"] = build_nc()
    return _NC_CACHE["nc"]


def _host_prep(x, W1, b1, W2, b2):
    x = np.ascontiguousarray(np.asarray(x, dtype=np.float32))
    W1 = np.asarray(W1, dtype=np.float64)
    W2 = np.asarray(W2, dtype=np.float64)
    weff = (W2 @ W1)[0].astype(np.float32)  # [D]
    weffb = np.ascontiguousarray(np.broadcast_to(weff, (P, D)))
    in_maps = []
    for c in range(NCORES):
        shard = np.ascontiguousarray(x[:, :, c * VS : (c + 1) * VS, :])
        in_maps.append({"x": shard, "weffb": weffb})
    return in_maps


def _host_post(results):
    """Divide the unnormalized pooled sums by Z computed from the scores."""
    outs = []
    for r in results:
        U = r["out"].reshape(B, VS, D).astype(np.float64)
        sc = r["sc"].astype(np.float64)  # [B, NCHUNK, S, VC]
        Z = np.exp(sc).sum(axis=2).reshape(B, VS)  # [B, VS]
        outs.append((U / Z[..., None]).astype(np.float32))
    return np.concatenate(outs, axis=1)


def kernel(x, W1, b1, W2, b2):
    from concourse.bass_utils import run_bass_kernel_spmd

    in_maps = _host_prep(x, W1, b1, W2, b2)
    nc = _get_nc()
    res = run_bass_kernel_spmd(nc, in_maps, core_ids=list(range(NCORES)))
    return _host_post(res.results)
